# revision 6
# baseline (speedup 1.0000x reference)
"""Trainium2 Bass kernel for nn_MiniGRUConv2d4 (MinGRU 4-direction conv scan).

Problem (B=4, Cin=64, Cout4=256, H=W=256):
    u_c  = conv3x3(xs, w_c) + bn_c          for c in {z, h, s}   (Cout=256)
    z    = sigmoid(u_z); hh = u_h; s = sigmoid(u_s)
    split 256 channels into 4 groups of 64; group g scans
      g=0: over H fwd, g=1: over H rev, g=2: over W fwd, g=3: over W rev
      h_i = z_i*hh_i + (1-z_i)*h_{i-1}
    out  = sum_g s_g * h_g                  (B, 64, H, W)

Sharding (8 cores): core = (batch b, orientation o).
  o=0: natural image, conv channels 128..255 (groups 2,3: W-fwd / W-rev)
  o=1: transposed image (host transposes), channels 0..127 (groups 0,1:
       H-scan becomes W-scan in the transposed frame).

v2 layout: each conv PSUM tile holds ONE scan group (64 chans) in
row-split form — partitions = (row-half h, chan c), filled by two
concurrent M=64 matmuls (col-group tiling: tile_position (0,0) and
(0,64)) whose rhs streams come from different band rows. The scan then
runs at full 128-partition width directly on conv output; the v1
SBUF->SBUF DMA remap (2 MB/band, 62% of all DMA traffic, and the sync
-queue head-of-line blocker) is gone. The conv itself is 5 K=128 fp16
matmul waves per (j, conv, group) tile (3x3 taps: dy0/dy1 pairs packed
into the 128-partition contraction via a row-shifted second copy of the
input; dy2 row rides partition-packed dx0/dx1 + a zero-top-half dx2
slot). Host folds BN into weights/biases, pads, transposes, and sums
the per-core partial products.
"""

import sys
import types

import numpy as np

import concourse.bass as bass
import concourse.mybir as mybir
import concourse.tile as tile

F32 = mybir.dt.float32
AF = mybir.ActivationFunctionType
OP = mybir.AluOpType

_R = 8  # band height (output rows per band)


# ---------------------------------------------------------------------------
# Workaround: the pinned walrus rejects instructions carrying more than a
# couple of sem waits ("Too many sync wait commands", CoreV3GenImpl
# setupSyncWait). Hoist excess waits onto same-engine NOPs inserted right
# before the offending instruction.
_MAX_WAITS = 1


def _split_excess_waits(nc, max_waits=_MAX_WAITS):
    import bass_rust

    n_split = 0
    for f in nc.m.functions:
        for blk in f.blocks:
            out = []
            for inst in blk.instructions:
                si = inst.sync_info
                if si is not None and len(si.on_wait) > max_waits:
                    waits = list(si.on_wait)
                    extra, keep = waits[:-max_waits], waits[-max_waits:]
                    for i0 in range(0, len(extra), max_waits):
                        nop = mybir.InstNoOp(
                            name=f"{inst.name}_xw{i0}", ins=[], outs=[]
                        )
                        nop.engine = inst.engine
                        nop.sync_info = bass_rust.SyncInfo(
                            on_wait=extra[i0 : i0 + max_waits], on_update=[]
                        )
                        nc.register_instruction(nop)
                        out.append(nop)
                        n_split += 1
                    inst.sync_info = bass_rust.SyncInfo(
                        on_wait=keep, on_update=list(si.on_update)
                    )
                out.append(inst)
            blk.instructions = out
    return n_split


def _ensure_axon_hooks_importable():
    # bass_utils imports antenv.axon_hooks when tracing is requested; the
    # container's antenv stub lacks it. Provide a no-op registry so the
    # import never crashes (tracing then just degrades gracefully).
    try:
        import antenv.axon_hooks  # noqa: F401
    except Exception:
        try:
            import antenv

            mod = types.ModuleType("antenv.axon_hooks")
            mod._hook = None
            mod.set_axon_ntff_profile_hook = lambda h: setattr(mod, "_hook", h)
            mod.get_axon_ntff_profile_hook = lambda: mod._hook
            sys.modules["antenv.axon_hooks"] = mod
            antenv.axon_hooks = mod
        except Exception:
            pass


# ---------------------------------------------------------------------------
# Device program

# Conv operands: fp16 runs the PE at full rate (1 cyc/row, like bf16) but
# carries a 10-bit mantissa — conv error ~5e-4 vs bf16's ~3e-3. fp32r would
# be exact-ish but its fused 4-byte weight load can't pipeline.
CONV_DT = mybir.dt.float16
CHAIN_DT = mybir.dt.float16  # z/s/a/b/h/p tiles + output (host upcasts)
WPOOL_BUFS = 4
XPOOL_BUFS = 3


def build_nc(H, W, with_init_fixup=True):
    """One-core program; all 8 cores run it SPMD with different inputs."""
    R = _R
    Rh = R // 2  # rows per half-band
    RR = R + 1  # input rows resident per band (dy0/dy1 buffer)
    Wp = W + 2
    assert H % R == 0 and W % 2 == 0
    nbands = H // R
    FW = Rh * W  # free width of one half-band slab (per partition)
    cdt = CONV_DT
    wdt = CHAIN_DT

    nc = bass.Bass("TRN2", target_bir_lowering=False, debug=False)
    xp = nc.dram_tensor("xp", [64, H + 2, Wp], cdt, kind="ExternalInput").ap()
    wts = nc.dram_tensor("wts", [128, 15, 128], cdt, kind="ExternalInput").ap()
    consts = nc.dram_tensor("consts", [128, 8], F32, kind="ExternalInput").ap()
    # out free dim: [band][group][half-rows x W]; partitions = (half, chan)
    out = nc.dram_tensor("out", [128, 2 * H * W // 128 * 64], wdt,
                         kind="ExternalOutput").ap()
    # 2*H*W*64/128 = H*W: per-partition free size is nbands * 2 * FW = H*W

    with tile.TileContext(nc) as tc:
        with (
            tc.tile_pool(name="const", bufs=1) as cpool,
            tc.tile_pool(name="xin", bufs=XPOOL_BUFS) as xpool,
            tc.tile_pool(name="work", bufs=WPOOL_BUFS) as wpool,
            tc.tile_pool(name="psum", bufs=2, space="PSUM") as ppool,
        ):
            wts_sb = cpool.tile([128, 15, 128], cdt)
            nc.sync.dma_start(wts_sb[:], wts)
            cst = cpool.tile([128, 8], F32)
            nc.sync.dma_start(cst[:], consts)
            # per-group bias vectors in (half, chan) layout
            bias = [[cst[:, 3 * g + c : 3 * g + c + 1] for c in range(3)]
                    for g in range(2)]  # bias[g][conv]
            init = [cst[:, 6 + g : 7 + g] for g in range(2)]

            for band in range(nbands):
                y0 = band * R
                # x2: dy0 rows at partitions 0:64, dy1 rows at 64:128
                x2 = xpool.tile([128, RR, Wp], cdt)
                nc.sync.dma_start(x2[0:64], xp[:, y0 : y0 + RR, :])
                nc.sync.dma_start(x2[64:128], xp[:, y0 + 1 : y0 + 1 + RR, :])
                # x3: dy2 rows; lower = col+0, upper = col+1. (Upper's last
                # column is never loaded or read.)
                x3 = xpool.tile([128, R, Wp], cdt)
                nc.sync.dma_start(x3[0:64], xp[:, y0 + 2 : y0 + 2 + R, :])
                nc.sync.dma_start(
                    x3[64:128, :, 0 : Wp - 1], xp[:, y0 + 2 : y0 + 2 + R, 1:Wp]
                )

                # per-group work tiles, all in (half, chan) partition layout;
                # free dim = Rh rows x W cols, row-major
                z_t = [wpool.tile([128, FW], wdt, name=f"z{g}") for g in range(2)]
                s_t = [wpool.tile([128, FW], wdt, name=f"s{g}") for g in range(2)]
                # [a | b] contiguous so the scan reads one tile
                ab_t = [wpool.tile([128, 2 * FW], wdt, name=f"ab{g}") for g in range(2)]
                h_t = [wpool.tile([128, FW], wdt, name=f"h{g}") for g in range(2)]
                # p for both groups in one tile -> single out DMA per band
                p_t = wpool.tile([128, 2 * FW], wdt)

                for j in range(2):  # j covers rows (2j, 2j+1) of each half
                    sl = slice(j * 2 * W, (j + 1) * 2 * W)
                    # K=128-dense waves for all 6 (group, conv) PSUM tiles
                    us = [[None] * 3 for _ in range(2)]
                    for g in range(2):  # scan group (fwd / bwd)
                        m0, m1 = 64 * g, 64 * g + 64
                        for c in range(3):  # z, h, s convs
                            u = ppool.tile(
                                [128, 2 * W], F32, name=f"u{c}g{g}",
                                tag=f"u{c}g{g}", bufs=(2 if c == 1 else 1),
                            )
                            us[g][c] = u
                            for hh in range(2):  # concurrent col-group halves
                                r0 = hh * Rh + 2 * j
                                p0, p1 = 64 * hh, 64 * hh + 64
                                for dx in range(3):  # dy0+dy1 pairs
                                    nc.tensor.matmul(
                                        u[p0:p1],
                                        wts_sb[:, 3 * c + dx, m0:m1],
                                        x2[:, r0 : r0 + 2, dx : dx + W],
                                        start=(dx == 0),
                                        stop=False,
                                    )
                                # dy2 (dx0, dx1) pair
                                nc.tensor.matmul(
                                    u[p0:p1],
                                    wts_sb[:, 9 + c, m0:m1],
                                    x3[:, r0 : r0 + 2, 0:W],
                                    start=False,
                                    stop=False,
                                )
                    # dy2 dx2 leftovers: K=64 matmuls. x3 holds this tap in
                    # BOTH partition halves (lower at col+2, upper at col+1),
                    # so two (group, conv) tiles pair on disjoint row groups
                    # and run concurrently (4 disjoint 64x64 quadrants/wave).
                    for i, (g, c) in enumerate(
                        (g, c) for g in range(2) for c in range(3)
                    ):
                        m0, m1 = 64 * g, 64 * g + 64
                        k0 = 64 * (i % 2)  # alternate low/up row half
                        xoff = 2 - (i % 2)  # low: col+2, up: col+1
                        for hh in range(2):
                            r0 = hh * Rh + 2 * j
                            p0, p1 = 64 * hh, 64 * hh + 64
                            nc.tensor.matmul(
                                us[g][c][p0:p1],
                                wts_sb[k0 : k0 + 64, 12 + c, m0:m1],
                                x3[k0 : k0 + 64, r0 : r0 + 2, xoff : xoff + W],
                                start=False,
                                stop=True,
                            )
                    for g in range(2):
                        nc.scalar.activation(
                            z_t[g][:, sl], us[g][0][:], AF.Sigmoid,
                            bias=bias[g][0],
                        )
                        nc.scalar.activation(
                            s_t[g][:, sl], us[g][2][:], AF.Sigmoid,
                            bias=bias[g][2],
                        )
                        # b = (u_h + bias_h) * z
                        nc.vector.scalar_tensor_tensor(
                            ab_t[g][:, FW + j * 2 * W : FW + (j + 1) * 2 * W],
                            us[g][1][:], bias[g][1], z_t[g][:, sl],
                            op0=OP.add, op1=OP.mult,
                        )

                for g in range(2):
                    a_f = ab_t[g][:, 0:FW]
                    b_f = ab_t[g][:, FW : 2 * FW]
                    # a = 1 - z (on ACT: Identity(-z + 1))
                    nc.scalar.activation(
                        a_f, z_t[g][:], AF.Identity, bias=1.0, scale=-1.0
                    )
                    a3 = a_f.rearrange("p (r w) -> p r w", w=W)
                    b3 = b_f.rearrange("p (r w) -> p r w", w=W)
                    edge = 0 if g == 0 else W - 1
                    # fold the (normally zero) scan init into b at each row
                    # edge, then zero `a` there so the flat scan restarts
                    # per row.
                    if with_init_fixup:
                        nc.vector.scalar_tensor_tensor(
                            b3[:, :, edge], a3[:, :, edge], init[g],
                            b3[:, :, edge], op0=OP.mult, op1=OP.add,
                        )
                    nc.scalar.activation(
                        a3[:, :, edge], a3[:, :, edge], AF.Copy,
                        bias=0.0, scale=0.0,
                    )
                    # scan at full 128-partition width; group 1 scans
                    # backward via reversed APs
                    if g == 0:
                        nc.vector.tensor_tensor_scan(
                            h_t[g][:, :], a_f, b_f, 0.0,
                            op0=OP.mult, op1=OP.add,
                        )
                    else:
                        nc.vector.tensor_tensor_scan(
                            h_t[g][:, ::-1], a_f[:, ::-1], b_f[:, ::-1], 0.0,
                            op0=OP.mult, op1=OP.add,
                        )
                    nc.vector.tensor_mul(
                        p_t[:, g * FW : (g + 1) * FW], s_t[g][:], h_t[g][:]
                    )
                # out store on the (otherwise idle) gpsimd software-DGE
                # queue: keeps the sync queue free of head-of-line blocking
                # behind the scan chain.
                nc.gpsimd.dma_start(
                    out[:, band * 2 * FW : (band + 1) * 2 * FW], p_t[:]
                )
    _split_excess_waits(nc)
    return nc


# ---------------------------------------------------------------------------
# Host side

_NC_CACHE = {}


def _get_nc(H, W, with_init_fixup=True):
    key = (H, W, with_init_fixup)
    if key not in _NC_CACHE:
        _NC_CACHE[key] = build_nc(H, W, with_init_fixup)
    return _NC_CACHE[key]


def make_in_maps(inputs, H, W):
    """Build the 8 per-core input dicts from the full problem inputs."""
    xs = np.ascontiguousarray(np.asarray(inputs["xs"], dtype=np.float32))
    B = xs.shape[0]
    Ws, Bs = {}, {}
    for tag in ("z", "h", "s"):
        w = np.asarray(inputs["w_" + tag], dtype=np.float32)
        g = np.asarray(inputs["g_" + tag], dtype=np.float32)
        be = np.asarray(inputs["b_" + tag], dtype=np.float32)
        m = np.asarray(inputs["m_" + tag], dtype=np.float32)
        v = np.asarray(inputs["v_" + tag], dtype=np.float32)
        inv = g / np.sqrt(v + 1e-5)
        Ws[tag] = w * inv[:, None, None, None]
        Bs[tag] = be - m * inv
    init = {
        k: np.asarray(inputs[k], dtype=np.float32).reshape(-1)
        for k in ("h20", "h21", "h30", "h31")
    }

    in_maps = []
    for b in range(B):
        for orient in (0, 1):
            if orient == 0:
                img = xs[b]
                ch = slice(128, 256)
                init_a, init_b = init["h30"], init["h31"]
            else:
                img = xs[b].transpose(0, 2, 1)
                ch = slice(0, 128)
                init_a, init_b = init["h20"], init["h21"]
            xpad = np.pad(img, ((0, 0), (1, 1), (1, 1)))
            wts = np.zeros((128, 15, 128), np.float32)
            consts = np.zeros((128, 8), np.float32)
            for c, tag in enumerate(("z", "h", "s")):
                wc = Ws[tag][ch]  # (128, 64, 3, 3) [cout, cin, ky, kx]
                if orient == 1:
                    wc = wc.transpose(0, 1, 3, 2)
                for dx in range(3):
                    wts[0:64, 3 * c + dx, :] = wc[:, :, 0, dx].T
                    wts[64:128, 3 * c + dx, :] = wc[:, :, 1, dx].T
                wts[0:64, 9 + c, :] = wc[:, :, 2, 0].T
                wts[64:128, 9 + c, :] = wc[:, :, 2, 1].T
                # dy2 dx2 weights in BOTH halves: the K=64 leftover matmuls
                # read it from either row half (paired for concurrency)
                wts[0:64, 12 + c, :] = wc[:, :, 2, 2].T
                wts[64:128, 12 + c, :] = wc[:, :, 2, 2].T
                # biases in (half, chan) layout, separate per scan group
                bg = Bs[tag][ch]
                consts[0:64, c] = bg[0:64]
                consts[64:128, c] = bg[0:64]
                consts[0:64, 3 + c] = bg[64:128]
                consts[64:128, 3 + c] = bg[64:128]
            consts[0:64, 6] = init_a
            consts[64:128, 6] = init_a
            consts[0:64, 7] = init_b
            consts[64:128, 7] = init_b
            cnp = mybir.dt.np(CONV_DT)
            if xpad.dtype != cnp:
                xpad = xpad.astype(cnp)
                wts = wts.astype(cnp)
            in_maps.append(
                {
                    "xp": np.ascontiguousarray(xpad),
                    "wts": wts,
                    "consts": consts,
                }
            )
    return in_maps


def gather_output(core_outs, B, H, W):
    """core_outs: list of 8 arrays (128, H*W) in core order (b-major).

    Device layout: partitions = (half hh in {0,1}, chan c in 0..63);
    free = [band][group g in {0,1}][row r in 0..Rh-1][col]. Global row of
    an element = band*R + hh*Rh + r.
    """
    R, Rh = _R, _R // 2
    nb = H // R
    out = np.empty((B, 64, H, W), np.float32)
    for b in range(B):
        for orient in (0, 1):
            o = core_outs[2 * b + orient].astype(np.float32)
            o = o.reshape(2, 64, nb, 2, Rh, W)  # hh, c, band, g, r, w
            o = o.sum(axis=3)  # sum the two scan groups: hh, c, band, r, w
            o = o.transpose(1, 2, 0, 3, 4).reshape(64, H, W)
            if orient == 0:
                out[b] = o
            else:
                out[b] += o.transpose(0, 2, 1)
    return out


def kernel(**inputs):
    from concourse.bass_utils import run_bass_kernel_spmd

    _ensure_axon_hooks_importable()
    xs = inputs["xs"]
    B, C, H, W = xs.shape
    # the scan-init fixup ops are only needed for nonzero initial states
    # (the problem spec ships all-zero inits)
    need_fixup = any(
        np.any(np.asarray(inputs[k], dtype=np.float32))
        for k in ("h20", "h21", "h30", "h31")
    )
    nc = _get_nc(H, W, with_init_fixup=need_fixup)
    in_maps = make_in_maps(inputs, H, W)
    res = run_bass_kernel_spmd(nc, in_maps, core_ids=list(range(len(in_maps))))
    outs = [res.results[c]["out"] for c in range(len(in_maps))]
    return gather_output(outs, B, H, W)


# revision 7
# speedup vs baseline: 1.0205x; 1.0205x over previous
"""Trainium2 Bass kernel for nn_MiniGRUConv2d4 (MinGRU 4-direction conv scan).

Problem (B=4, Cin=64, Cout4=256, H=W=256):
    u_c  = conv3x3(xs, w_c) + bn_c          for c in {z, h, s}   (Cout=256)
    z    = sigmoid(u_z); hh = u_h; s = sigmoid(u_s)
    split 256 channels into 4 groups of 64; group g scans
      g=0: over H fwd, g=1: over H rev, g=2: over W fwd, g=3: over W rev
      h_i = z_i*hh_i + (1-z_i)*h_{i-1}
    out  = sum_g s_g * h_g                  (B, 64, H, W)

Sharding (8 cores): core = (batch b, orientation o).
  o=0: natural image, conv channels 128..255 (groups 2,3: W-fwd / W-rev)
  o=1: transposed image (host transposes), channels 0..127 (groups 0,1:
       H-scan becomes W-scan in the transposed frame).

v2 layout: each conv PSUM tile holds ONE scan group (64 chans) in
row-split form — partitions = (row-half h, chan c), filled by two
concurrent M=64 matmuls (col-group tiling: tile_position (0,0) and
(0,64)) whose rhs streams come from different band rows. The scan then
runs at full 128-partition width directly on conv output; the v1
SBUF->SBUF DMA remap (2 MB/band, 62% of all DMA traffic, and the sync
-queue head-of-line blocker) is gone. The conv itself is 5 K=128 fp16
matmul waves per (j, conv, group) tile (3x3 taps: dy0/dy1 pairs packed
into the 128-partition contraction via a row-shifted second copy of the
input; dy2 row rides partition-packed dx0/dx1 + a zero-top-half dx2
slot). Host folds BN into weights/biases, pads, transposes, and sums
the per-core partial products.
"""

import sys
import types

import numpy as np

import concourse.bass as bass
import concourse.mybir as mybir
import concourse.tile as tile

F32 = mybir.dt.float32
AF = mybir.ActivationFunctionType
OP = mybir.AluOpType

_R = 8  # band height (output rows per band)


# ---------------------------------------------------------------------------
# Workaround: the pinned walrus rejects instructions carrying more than a
# couple of sem waits ("Too many sync wait commands", CoreV3GenImpl
# setupSyncWait). Hoist excess waits onto same-engine NOPs inserted right
# before the offending instruction.
_MAX_WAITS = 1


def _split_excess_waits(nc, max_waits=_MAX_WAITS):
    import bass_rust

    n_split = 0
    for f in nc.m.functions:
        for blk in f.blocks:
            out = []
            for inst in blk.instructions:
                si = inst.sync_info
                if si is not None and len(si.on_wait) > max_waits:
                    waits = list(si.on_wait)
                    extra, keep = waits[:-max_waits], waits[-max_waits:]
                    for i0 in range(0, len(extra), max_waits):
                        nop = mybir.InstNoOp(
                            name=f"{inst.name}_xw{i0}", ins=[], outs=[]
                        )
                        nop.engine = inst.engine
                        nop.sync_info = bass_rust.SyncInfo(
                            on_wait=extra[i0 : i0 + max_waits], on_update=[]
                        )
                        nc.register_instruction(nop)
                        out.append(nop)
                        n_split += 1
                    inst.sync_info = bass_rust.SyncInfo(
                        on_wait=keep, on_update=list(si.on_update)
                    )
                out.append(inst)
            blk.instructions = out
    return n_split


def _ensure_axon_hooks_importable():
    # bass_utils imports antenv.axon_hooks when tracing is requested; the
    # container's antenv stub lacks it. Provide a no-op registry so the
    # import never crashes (tracing then just degrades gracefully).
    try:
        import antenv.axon_hooks  # noqa: F401
    except Exception:
        try:
            import antenv

            mod = types.ModuleType("antenv.axon_hooks")
            mod._hook = None
            mod.set_axon_ntff_profile_hook = lambda h: setattr(mod, "_hook", h)
            mod.get_axon_ntff_profile_hook = lambda: mod._hook
            sys.modules["antenv.axon_hooks"] = mod
            antenv.axon_hooks = mod
        except Exception:
            pass


# ---------------------------------------------------------------------------
# Device program

# Conv operands: fp16 runs the PE at full rate (1 cyc/row, like bf16) but
# carries a 10-bit mantissa — conv error ~5e-4 vs bf16's ~3e-3. fp32r would
# be exact-ish but its fused 4-byte weight load can't pipeline.
CONV_DT = mybir.dt.float16
CHAIN_DT = mybir.dt.float16  # z/s/a/b/h/p tiles + output (host upcasts)
WPOOL_BUFS = 4
XPOOL_BUFS = 3


def build_nc(H, W, with_init_fixup=True):
    """One-core program; all 8 cores run it SPMD with different inputs."""
    R = _R
    Rh = R // 2  # rows per half-band
    RR = R + 1  # input rows resident per band (dy0/dy1 buffer)
    Wp = W + 2
    assert H % R == 0 and W % 2 == 0
    nbands = H // R
    FW = Rh * W  # free width of one half-band slab (per partition)
    cdt = CONV_DT
    wdt = CHAIN_DT

    nc = bass.Bass("TRN2", target_bir_lowering=False, debug=False)
    xp = nc.dram_tensor("xp", [64, H + 2, Wp], cdt, kind="ExternalInput").ap()
    wts = nc.dram_tensor("wts", [128, 15, 128], cdt, kind="ExternalInput").ap()
    consts = nc.dram_tensor("consts", [128, 8], F32, kind="ExternalInput").ap()
    # out free dim: [band][group][half-rows x W]; partitions = (half, chan)
    out = nc.dram_tensor("out", [128, 2 * H * W // 128 * 64], wdt,
                         kind="ExternalOutput").ap()
    # 2*H*W*64/128 = H*W: per-partition free size is nbands * 2 * FW = H*W

    with tile.TileContext(nc) as tc:
        with (
            tc.tile_pool(name="const", bufs=1) as cpool,
            tc.tile_pool(name="xin", bufs=XPOOL_BUFS) as xpool,
            tc.tile_pool(name="work", bufs=WPOOL_BUFS) as wpool,
            tc.tile_pool(name="psum", bufs=2, space="PSUM") as ppool,
        ):
            wts_sb = cpool.tile([128, 15, 128], cdt)
            nc.sync.dma_start(wts_sb[:], wts)
            cst = cpool.tile([128, 8], F32)
            nc.sync.dma_start(cst[:], consts)
            # per-group bias vectors in (half, chan) layout
            bias = [[cst[:, 3 * g + c : 3 * g + c + 1] for c in range(3)]
                    for g in range(2)]  # bias[g][conv]
            init = [cst[:, 6 + g : 7 + g] for g in range(2)]

            for band in range(nbands):
                y0 = band * R
                # x2: dy0 rows at partitions 0:64, dy1 rows at 64:128
                x2 = xpool.tile([128, RR, Wp], cdt)
                nc.sync.dma_start(x2[0:64], xp[:, y0 : y0 + RR, :])
                nc.sync.dma_start(x2[64:128], xp[:, y0 + 1 : y0 + 1 + RR, :])
                # x3: dy2 rows; lower = col+0, upper = col+1. (Upper's last
                # column is never loaded or read.)
                x3 = xpool.tile([128, R, Wp], cdt)
                nc.sync.dma_start(x3[0:64], xp[:, y0 + 2 : y0 + 2 + R, :])
                nc.sync.dma_start(
                    x3[64:128, :, 0 : Wp - 1], xp[:, y0 + 2 : y0 + 2 + R, 1:Wp]
                )

                # per-group work tiles, all in (half, chan) partition layout;
                # free dim = Rh rows x W cols, row-major
                z_t = [wpool.tile([128, FW], wdt, name=f"z{g}") for g in range(2)]
                s_t = [wpool.tile([128, FW], wdt, name=f"s{g}") for g in range(2)]
                # [a | b] contiguous so the scan reads one tile
                ab_t = [wpool.tile([128, 2 * FW], wdt, name=f"ab{g}") for g in range(2)]
                h_t = [wpool.tile([128, FW], wdt, name=f"h{g}") for g in range(2)]
                # p for both groups in one tile -> single out DMA per band
                p_t = wpool.tile([128, 2 * FW], wdt)

                for j in range(2):  # j covers rows (2j, 2j+1) of each half
                    sl = slice(j * 2 * W, (j + 1) * 2 * W)
                    # K=128-dense waves for all 6 (group, conv) PSUM tiles,
                    # with the dy2-dx2 leftovers emitted as paired K=64
                    # matmuls after every SECOND tile: x3 holds that tap in
                    # BOTH partition halves (lower at col+2, upper at
                    # col+1), so two tiles' leftovers land on disjoint row
                    # groups and run concurrently (4 disjoint 64x64
                    # quadrants per wave). Pairing mid-sequence (not at the
                    # end of the j-step) lets each PSUM tile stop early so
                    # its ACT/DVE consumer frees it before the next j-step
                    # needs the bank.
                    us = [[None] * 3 for _ in range(2)]
                    tiles = [(g, c) for g in range(2) for c in range(3)]
                    for i, (g, c) in enumerate(tiles):
                        m0, m1 = 64 * g, 64 * g + 64
                        u = ppool.tile(
                            [128, 2 * W], F32, name=f"u{c}g{g}",
                            tag=f"u{c}g{g}", bufs=(2 if c == 1 else 1),
                        )
                        us[g][c] = u
                        for hh in range(2):  # concurrent col-group halves
                            r0 = hh * Rh + 2 * j
                            p0, p1 = 64 * hh, 64 * hh + 64
                            for dx in range(3):  # dy0+dy1 pairs
                                nc.tensor.matmul(
                                    u[p0:p1],
                                    wts_sb[:, 3 * c + dx, m0:m1],
                                    x2[:, r0 : r0 + 2, dx : dx + W],
                                    start=(dx == 0),
                                    stop=False,
                                )
                            # dy2 (dx0, dx1) pair
                            nc.tensor.matmul(
                                u[p0:p1],
                                wts_sb[:, 9 + c, m0:m1],
                                x3[:, r0 : r0 + 2, 0:W],
                                start=False,
                                stop=False,
                            )
                        if i % 2 == 1:  # leftover pair-wave for tiles i-1, i
                            for k, (gg, cc) in enumerate(tiles[i - 1 : i + 1]):
                                mm0, mm1 = 64 * gg, 64 * gg + 64
                                k0 = 64 * k  # first: low half, second: up
                                xoff = 2 - k  # low: col+2, up: col+1
                                for hh in range(2):
                                    r0 = hh * Rh + 2 * j
                                    p0, p1 = 64 * hh, 64 * hh + 64
                                    nc.tensor.matmul(
                                        us[gg][cc][p0:p1],
                                        wts_sb[k0 : k0 + 64, 12 + cc, mm0:mm1],
                                        x3[k0 : k0 + 64, r0 : r0 + 2,
                                           xoff : xoff + W],
                                        start=False,
                                        stop=True,
                                    )
                    for g in range(2):
                        nc.scalar.activation(
                            z_t[g][:, sl], us[g][0][:], AF.Sigmoid,
                            bias=bias[g][0],
                        )
                        nc.scalar.activation(
                            s_t[g][:, sl], us[g][2][:], AF.Sigmoid,
                            bias=bias[g][2],
                        )
                        # b = (u_h + bias_h) * z
                        nc.vector.scalar_tensor_tensor(
                            ab_t[g][:, FW + j * 2 * W : FW + (j + 1) * 2 * W],
                            us[g][1][:], bias[g][1], z_t[g][:, sl],
                            op0=OP.add, op1=OP.mult,
                        )

                for g in range(2):
                    a_f = ab_t[g][:, 0:FW]
                    b_f = ab_t[g][:, FW : 2 * FW]
                    # a = 1 - z (on ACT: Identity(-z + 1))
                    nc.scalar.activation(
                        a_f, z_t[g][:], AF.Identity, bias=1.0, scale=-1.0
                    )
                    a3 = a_f.rearrange("p (r w) -> p r w", w=W)
                    b3 = b_f.rearrange("p (r w) -> p r w", w=W)
                    edge = 0 if g == 0 else W - 1
                    # fold the (normally zero) scan init into b at each row
                    # edge, then zero `a` there so the flat scan restarts
                    # per row.
                    if with_init_fixup:
                        nc.vector.scalar_tensor_tensor(
                            b3[:, :, edge], a3[:, :, edge], init[g],
                            b3[:, :, edge], op0=OP.mult, op1=OP.add,
                        )
                    nc.scalar.activation(
                        a3[:, :, edge], a3[:, :, edge], AF.Copy,
                        bias=0.0, scale=0.0,
                    )
                    # scan at full 128-partition width; group 1 scans
                    # backward via reversed APs
                    if g == 0:
                        nc.vector.tensor_tensor_scan(
                            h_t[g][:, :], a_f, b_f, 0.0,
                            op0=OP.mult, op1=OP.add,
                        )
                    else:
                        nc.vector.tensor_tensor_scan(
                            h_t[g][:, ::-1], a_f[:, ::-1], b_f[:, ::-1], 0.0,
                            op0=OP.mult, op1=OP.add,
                        )
                    nc.vector.tensor_mul(
                        p_t[:, g * FW : (g + 1) * FW], s_t[g][:], h_t[g][:]
                    )
                # out store on the (otherwise idle) gpsimd software-DGE
                # queue: keeps the sync queue free of head-of-line blocking
                # behind the scan chain.
                nc.gpsimd.dma_start(
                    out[:, band * 2 * FW : (band + 1) * 2 * FW], p_t[:]
                )
    _split_excess_waits(nc)
    return nc


# ---------------------------------------------------------------------------
# Host side

_NC_CACHE = {}


def _get_nc(H, W, with_init_fixup=True):
    key = (H, W, with_init_fixup)
    if key not in _NC_CACHE:
        _NC_CACHE[key] = build_nc(H, W, with_init_fixup)
    return _NC_CACHE[key]


def make_in_maps(inputs, H, W):
    """Build the 8 per-core input dicts from the full problem inputs."""
    xs = np.ascontiguousarray(np.asarray(inputs["xs"], dtype=np.float32))
    B = xs.shape[0]
    Ws, Bs = {}, {}
    for tag in ("z", "h", "s"):
        w = np.asarray(inputs["w_" + tag], dtype=np.float32)
        g = np.asarray(inputs["g_" + tag], dtype=np.float32)
        be = np.asarray(inputs["b_" + tag], dtype=np.float32)
        m = np.asarray(inputs["m_" + tag], dtype=np.float32)
        v = np.asarray(inputs["v_" + tag], dtype=np.float32)
        inv = g / np.sqrt(v + 1e-5)
        Ws[tag] = w * inv[:, None, None, None]
        Bs[tag] = be - m * inv
    init = {
        k: np.asarray(inputs[k], dtype=np.float32).reshape(-1)
        for k in ("h20", "h21", "h30", "h31")
    }

    in_maps = []
    for b in range(B):
        for orient in (0, 1):
            if orient == 0:
                img = xs[b]
                ch = slice(128, 256)
                init_a, init_b = init["h30"], init["h31"]
            else:
                img = xs[b].transpose(0, 2, 1)
                ch = slice(0, 128)
                init_a, init_b = init["h20"], init["h21"]
            xpad = np.pad(img, ((0, 0), (1, 1), (1, 1)))
            wts = np.zeros((128, 15, 128), np.float32)
            consts = np.zeros((128, 8), np.float32)
            for c, tag in enumerate(("z", "h", "s")):
                wc = Ws[tag][ch]  # (128, 64, 3, 3) [cout, cin, ky, kx]
                if orient == 1:
                    wc = wc.transpose(0, 1, 3, 2)
                for dx in range(3):
                    wts[0:64, 3 * c + dx, :] = wc[:, :, 0, dx].T
                    wts[64:128, 3 * c + dx, :] = wc[:, :, 1, dx].T
                wts[0:64, 9 + c, :] = wc[:, :, 2, 0].T
                wts[64:128, 9 + c, :] = wc[:, :, 2, 1].T
                # dy2 dx2 weights in BOTH halves: the K=64 leftover matmuls
                # read it from either row half (paired for concurrency)
                wts[0:64, 12 + c, :] = wc[:, :, 2, 2].T
                wts[64:128, 12 + c, :] = wc[:, :, 2, 2].T
                # biases in (half, chan) layout, separate per scan group
                bg = Bs[tag][ch]
                consts[0:64, c] = bg[0:64]
                consts[64:128, c] = bg[0:64]
                consts[0:64, 3 + c] = bg[64:128]
                consts[64:128, 3 + c] = bg[64:128]
            consts[0:64, 6] = init_a
            consts[64:128, 6] = init_a
            consts[0:64, 7] = init_b
            consts[64:128, 7] = init_b
            cnp = mybir.dt.np(CONV_DT)
            if xpad.dtype != cnp:
                xpad = xpad.astype(cnp)
                wts = wts.astype(cnp)
            in_maps.append(
                {
                    "xp": np.ascontiguousarray(xpad),
                    "wts": wts,
                    "consts": consts,
                }
            )
    return in_maps


def gather_output(core_outs, B, H, W):
    """core_outs: list of 8 arrays (128, H*W) in core order (b-major).

    Device layout: partitions = (half hh in {0,1}, chan c in 0..63);
    free = [band][group g in {0,1}][row r in 0..Rh-1][col]. Global row of
    an element = band*R + hh*Rh + r.
    """
    R, Rh = _R, _R // 2
    nb = H // R
    out = np.empty((B, 64, H, W), np.float32)
    for b in range(B):
        for orient in (0, 1):
            o = core_outs[2 * b + orient].astype(np.float32)
            o = o.reshape(2, 64, nb, 2, Rh, W)  # hh, c, band, g, r, w
            o = o.sum(axis=3)  # sum the two scan groups: hh, c, band, r, w
            o = o.transpose(1, 2, 0, 3, 4).reshape(64, H, W)
            if orient == 0:
                out[b] = o
            else:
                out[b] += o.transpose(0, 2, 1)
    return out


def kernel(**inputs):
    from concourse.bass_utils import run_bass_kernel_spmd

    _ensure_axon_hooks_importable()
    xs = inputs["xs"]
    B, C, H, W = xs.shape
    # the scan-init fixup ops are only needed for nonzero initial states
    # (the problem spec ships all-zero inits)
    need_fixup = any(
        np.any(np.asarray(inputs[k], dtype=np.float32))
        for k in ("h20", "h21", "h30", "h31")
    )
    nc = _get_nc(H, W, with_init_fixup=need_fixup)
    in_maps = make_in_maps(inputs, H, W)
    res = run_bass_kernel_spmd(nc, in_maps, core_ids=list(range(len(in_maps))))
    outs = [res.results[c]["out"] for c in range(len(in_maps))]
    return gather_output(outs, B, H, W)


# revision 9
# speedup vs baseline: 1.0872x; 1.0654x over previous
"""Trainium2 Bass kernel for nn_MiniGRUConv2d4 (MinGRU 4-direction conv scan).

Problem (B=4, Cin=64, Cout4=256, H=W=256):
    u_c  = conv3x3(xs, w_c) + bn_c          for c in {z, h, s}   (Cout=256)
    z    = sigmoid(u_z); hh = u_h; s = sigmoid(u_s)
    split 256 channels into 4 groups of 64; group g scans
      g=0: over H fwd, g=1: over H rev, g=2: over W fwd, g=3: over W rev
      h_i = z_i*hh_i + (1-z_i)*h_{i-1}
    out  = sum_g s_g * h_g                  (B, 64, H, W)

Sharding (8 cores): core = (batch b, orientation o).
  o=0: natural image, conv channels 128..255 (groups 2,3: W-fwd / W-rev)
  o=1: transposed image (host transposes), channels 0..127 (groups 0,1:
       H-scan becomes W-scan in the transposed frame).

v2 layout: each conv PSUM tile holds ONE scan group (64 chans) in
row-split form — partitions = (row-half h, chan c), filled by two
concurrent M=64 matmuls (col-group tiling: tile_position (0,0) and
(0,64)) whose rhs streams come from different band rows. The scan then
runs at full 128-partition width directly on conv output; the v1
SBUF->SBUF DMA remap (2 MB/band, 62% of all DMA traffic, and the sync
-queue head-of-line blocker) is gone. The conv itself is 5 K=128 fp16
matmul waves per (j, conv, group) tile (3x3 taps: dy0/dy1 pairs packed
into the 128-partition contraction via a row-shifted second copy of the
input; dy2 row rides partition-packed dx0/dx1 + a zero-top-half dx2
slot). Host folds BN into weights/biases, pads, transposes, and sums
the per-core partial products.
"""

import sys
import types

import numpy as np

import concourse.bass as bass
import concourse.mybir as mybir
import concourse.tile as tile

F32 = mybir.dt.float32
AF = mybir.ActivationFunctionType
OP = mybir.AluOpType

_R = 8  # band height (output rows per band)


# ---------------------------------------------------------------------------
# Workaround: the pinned walrus rejects instructions carrying more than a
# couple of sem waits ("Too many sync wait commands", CoreV3GenImpl
# setupSyncWait). Hoist excess waits onto same-engine NOPs inserted right
# before the offending instruction.
_MAX_WAITS = 1


def _split_excess_waits(nc, max_waits=_MAX_WAITS):
    import bass_rust

    n_split = 0
    for f in nc.m.functions:
        for blk in f.blocks:
            out = []
            for inst in blk.instructions:
                si = inst.sync_info
                if si is not None and len(si.on_wait) > max_waits:
                    waits = list(si.on_wait)
                    extra, keep = waits[:-max_waits], waits[-max_waits:]
                    for i0 in range(0, len(extra), max_waits):
                        nop = mybir.InstNoOp(
                            name=f"{inst.name}_xw{i0}", ins=[], outs=[]
                        )
                        nop.engine = inst.engine
                        nop.sync_info = bass_rust.SyncInfo(
                            on_wait=extra[i0 : i0 + max_waits], on_update=[]
                        )
                        nc.register_instruction(nop)
                        out.append(nop)
                        n_split += 1
                    inst.sync_info = bass_rust.SyncInfo(
                        on_wait=keep, on_update=list(si.on_update)
                    )
                out.append(inst)
            blk.instructions = out
    return n_split


def _ensure_axon_hooks_importable():
    # bass_utils imports antenv.axon_hooks when tracing is requested; the
    # container's antenv stub lacks it. Provide a no-op registry so the
    # import never crashes (tracing then just degrades gracefully).
    try:
        import antenv.axon_hooks  # noqa: F401
    except Exception:
        try:
            import antenv

            mod = types.ModuleType("antenv.axon_hooks")
            mod._hook = None
            mod.set_axon_ntff_profile_hook = lambda h: setattr(mod, "_hook", h)
            mod.get_axon_ntff_profile_hook = lambda: mod._hook
            sys.modules["antenv.axon_hooks"] = mod
            antenv.axon_hooks = mod
        except Exception:
            pass


# ---------------------------------------------------------------------------
# Device program

# Conv operands: fp16 runs the PE at full rate (1 cyc/row, like bf16) but
# carries a 10-bit mantissa — conv error ~5e-4 vs bf16's ~3e-3. fp32r would
# be exact-ish but its fused 4-byte weight load can't pipeline.
CONV_DT = mybir.dt.float16
CHAIN_DT = mybir.dt.float16  # z/s/a/b/h/p tiles + output (host upcasts)
WPOOL_BUFS = 4
XPOOL_BUFS = 3


def build_nc(H, W, with_init_fixup=True):
    """One-core program; all 8 cores run it SPMD with different inputs."""
    R = _R
    Rh = R // 2  # rows per half-band
    RR = R + 1  # input rows resident per band (dy0/dy1 buffer)
    Wp = W + 2
    assert H % R == 0 and W % 2 == 0
    nbands = H // R
    FW = Rh * W  # free width of one half-band slab (per partition)
    cdt = CONV_DT
    wdt = CHAIN_DT

    nc = bass.Bass("TRN2", target_bir_lowering=False, debug=False)
    xp = nc.dram_tensor("xp", [64, H + 2, Wp], cdt, kind="ExternalInput").ap()
    wts = nc.dram_tensor("wts", [128, 15, 128], cdt, kind="ExternalInput").ap()
    consts = nc.dram_tensor("consts", [128, 8], F32, kind="ExternalInput").ap()
    # out free dim: [band][group][half-rows x W]; partitions = (half, chan)
    out = nc.dram_tensor("out", [128, 2 * H * W // 128 * 64], wdt,
                         kind="ExternalOutput").ap()
    # 2*H*W*64/128 = H*W: per-partition free size is nbands * 2 * FW = H*W

    with tile.TileContext(nc) as tc:
        with (
            tc.tile_pool(name="const", bufs=1) as cpool,
            tc.tile_pool(name="xin", bufs=XPOOL_BUFS) as xpool,
            tc.tile_pool(name="work", bufs=WPOOL_BUFS) as wpool,
            tc.tile_pool(name="psum", bufs=2, space="PSUM") as ppool,
        ):
            wts_sb = cpool.tile([128, 15, 128], cdt)
            nc.sync.dma_start(wts_sb[:], wts)
            cst = cpool.tile([128, 8], F32)
            nc.sync.dma_start(cst[:], consts)
            # per-group bias vectors in (half, chan) layout
            bias = [[cst[:, 3 * g + c : 3 * g + c + 1] for c in range(3)]
                    for g in range(2)]  # bias[g][conv]
            init = [cst[:, 6 + g : 7 + g] for g in range(2)]

            for band in range(nbands):
                y0 = band * R
                # x2: dy0 rows at partitions 0:64, dy1 rows at 64:128
                x2 = xpool.tile([128, RR, Wp], cdt)
                nc.sync.dma_start(x2[0:64], xp[:, y0 : y0 + RR, :])
                nc.sync.dma_start(x2[64:128], xp[:, y0 + 1 : y0 + 1 + RR, :])
                # x3: dy2 rows; lower = col+0, upper = col+1. (Upper's last
                # column is never loaded or read.)
                x3 = xpool.tile([128, R, Wp], cdt)
                nc.sync.dma_start(x3[0:64], xp[:, y0 + 2 : y0 + 2 + R, :])
                nc.sync.dma_start(
                    x3[64:128, :, 0 : Wp - 1], xp[:, y0 + 2 : y0 + 2 + R, 1:Wp]
                )

                # per-group work tiles, all in (half, chan) partition layout;
                # free dim = Rh rows x W cols, row-major
                z_t = [wpool.tile([128, FW], wdt, name=f"z{g}") for g in range(2)]
                s_t = [wpool.tile([128, FW], wdt, name=f"s{g}") for g in range(2)]
                # [a | b] contiguous so the scan reads one tile
                ab_t = [wpool.tile([128, 2 * FW], wdt, name=f"ab{g}") for g in range(2)]
                h_t = [wpool.tile([128, FW], wdt, name=f"h{g}") for g in range(2)]
                # p for both groups in one tile -> single out DMA per band
                p_t = wpool.tile([128, 2 * FW], wdt)

                for j in range(2):  # j covers rows (2j, 2j+1) of each half
                    sl = slice(j * 2 * W, (j + 1) * 2 * W)
                    # K=128-dense waves for all 6 (group, conv) PSUM tiles,
                    # with the dy2-dx2 leftovers emitted as paired K=64
                    # matmuls after every SECOND tile: x3 holds that tap in
                    # BOTH partition halves (lower at col+2, upper at
                    # col+1), so two tiles' leftovers land on disjoint row
                    # groups and run concurrently (4 disjoint 64x64
                    # quadrants per wave). Pairing mid-sequence (not at the
                    # end of the j-step) lets each PSUM tile stop early so
                    # its ACT/DVE consumer frees it before the next j-step
                    # needs the bank.
                    us = [[None] * 3 for _ in range(2)]
                    # Tile order chosen so the NEXT j-step's bank reuse
                    # never stalls: it starts with the double-buffered
                    # h-conv tiles, and the single-buffered tiles' consumers
                    # (emitted in matching order below) finish before their
                    # banks are needed again.
                    tiles = [(0, 1), (0, 2), (0, 0), (1, 1), (1, 2), (1, 0)]
                    for g, c in tiles:
                        m0, m1 = 64 * g, 64 * g + 64
                        u = ppool.tile(
                            [128, 2 * W], F32, name=f"u{c}g{g}",
                            tag=f"u{c}g{g}", bufs=(2 if c == 1 else 1),
                        )
                        us[g][c] = u
                        for hh in range(2):  # concurrent col-group halves
                            r0 = hh * Rh + 2 * j
                            p0, p1 = 64 * hh, 64 * hh + 64
                            for dx in range(3):  # dy0+dy1 pairs
                                nc.tensor.matmul(
                                    u[p0:p1],
                                    wts_sb[:, 3 * c + dx, m0:m1],
                                    x2[:, r0 : r0 + 2, dx : dx + W],
                                    start=(dx == 0),
                                    stop=False,
                                )
                            # dy2 (dx0, dx1) pair
                            nc.tensor.matmul(
                                u[p0:p1],
                                wts_sb[:, 9 + c, m0:m1],
                                x3[:, r0 : r0 + 2, 0:W],
                                start=False,
                                stop=False,
                            )
                    # dy2-dx2 leftovers as one grouped block of 3 pair-waves
                    # (K=64, 4 disjoint quadrants each): grouping pays the
                    # row-tiled LDWEIGHTS drain-stall once instead of per
                    # pair.
                    for i in range(0, 6, 2):
                        for k, (gg, cc) in enumerate(tiles[i : i + 2]):
                            mm0, mm1 = 64 * gg, 64 * gg + 64
                            k0 = 64 * k  # first: low half, second: up
                            xoff = 2 - k  # low: col+2, up: col+1
                            for hh in range(2):
                                r0 = hh * Rh + 2 * j
                                p0, p1 = 64 * hh, 64 * hh + 64
                                nc.tensor.matmul(
                                    us[gg][cc][p0:p1],
                                    wts_sb[k0 : k0 + 64, 12 + cc, mm0:mm1],
                                    x3[k0 : k0 + 64, r0 : r0 + 2,
                                       xoff : xoff + W],
                                    start=False,
                                    stop=True,
                                )
                    # consumers in PSUM-availability order (s-g0's bank is
                    # the first one the next j-step reuses)
                    for g in range(2):
                        nc.scalar.activation(
                            s_t[g][:, sl], us[g][2][:], AF.Sigmoid,
                            bias=bias[g][2],
                        )
                        nc.scalar.activation(
                            z_t[g][:, sl], us[g][0][:], AF.Sigmoid,
                            bias=bias[g][0],
                        )
                        # b = (u_h + bias_h) * z
                        nc.vector.scalar_tensor_tensor(
                            ab_t[g][:, FW + j * 2 * W : FW + (j + 1) * 2 * W],
                            us[g][1][:], bias[g][1], z_t[g][:, sl],
                            op0=OP.add, op1=OP.mult,
                        )

                for g in range(2):
                    a_f = ab_t[g][:, 0:FW]
                    b_f = ab_t[g][:, FW : 2 * FW]
                    # a = 1 - z (on ACT: Identity(-z + 1))
                    nc.scalar.activation(
                        a_f, z_t[g][:], AF.Identity, bias=1.0, scale=-1.0
                    )
                    a3 = a_f.rearrange("p (r w) -> p r w", w=W)
                    b3 = b_f.rearrange("p (r w) -> p r w", w=W)
                    edge = 0 if g == 0 else W - 1
                    # fold the (normally zero) scan init into b at each row
                    # edge, then zero `a` there so the flat scan restarts
                    # per row.
                    if with_init_fixup:
                        nc.vector.scalar_tensor_tensor(
                            b3[:, :, edge], a3[:, :, edge], init[g],
                            b3[:, :, edge], op0=OP.mult, op1=OP.add,
                        )
                    nc.scalar.activation(
                        a3[:, :, edge], a3[:, :, edge], AF.Copy,
                        bias=0.0, scale=0.0,
                    )
                    # scan at full 128-partition width; group 1 scans
                    # backward via reversed APs
                    if g == 0:
                        nc.vector.tensor_tensor_scan(
                            h_t[g][:, :], a_f, b_f, 0.0,
                            op0=OP.mult, op1=OP.add,
                        )
                    else:
                        nc.vector.tensor_tensor_scan(
                            h_t[g][:, ::-1], a_f[:, ::-1], b_f[:, ::-1], 0.0,
                            op0=OP.mult, op1=OP.add,
                        )
                    nc.vector.tensor_mul(
                        p_t[:, g * FW : (g + 1) * FW], s_t[g][:], h_t[g][:]
                    )
                # out store on the (otherwise idle) gpsimd software-DGE
                # queue: keeps the sync queue free of head-of-line blocking
                # behind the scan chain.
                nc.gpsimd.dma_start(
                    out[:, band * 2 * FW : (band + 1) * 2 * FW], p_t[:]
                )
    _split_excess_waits(nc)
    return nc


# ---------------------------------------------------------------------------
# Host side

_NC_CACHE = {}


def _get_nc(H, W, with_init_fixup=True):
    key = (H, W, with_init_fixup)
    if key not in _NC_CACHE:
        _NC_CACHE[key] = build_nc(H, W, with_init_fixup)
    return _NC_CACHE[key]


def make_in_maps(inputs, H, W):
    """Build the 8 per-core input dicts from the full problem inputs."""
    xs = np.ascontiguousarray(np.asarray(inputs["xs"], dtype=np.float32))
    B = xs.shape[0]
    Ws, Bs = {}, {}
    for tag in ("z", "h", "s"):
        w = np.asarray(inputs["w_" + tag], dtype=np.float32)
        g = np.asarray(inputs["g_" + tag], dtype=np.float32)
        be = np.asarray(inputs["b_" + tag], dtype=np.float32)
        m = np.asarray(inputs["m_" + tag], dtype=np.float32)
        v = np.asarray(inputs["v_" + tag], dtype=np.float32)
        inv = g / np.sqrt(v + 1e-5)
        Ws[tag] = w * inv[:, None, None, None]
        Bs[tag] = be - m * inv
    init = {
        k: np.asarray(inputs[k], dtype=np.float32).reshape(-1)
        for k in ("h20", "h21", "h30", "h31")
    }

    in_maps = []
    for b in range(B):
        for orient in (0, 1):
            if orient == 0:
                img = xs[b]
                ch = slice(128, 256)
                init_a, init_b = init["h30"], init["h31"]
            else:
                img = xs[b].transpose(0, 2, 1)
                ch = slice(0, 128)
                init_a, init_b = init["h20"], init["h21"]
            xpad = np.pad(img, ((0, 0), (1, 1), (1, 1)))
            wts = np.zeros((128, 15, 128), np.float32)
            consts = np.zeros((128, 8), np.float32)
            for c, tag in enumerate(("z", "h", "s")):
                wc = Ws[tag][ch]  # (128, 64, 3, 3) [cout, cin, ky, kx]
                if orient == 1:
                    wc = wc.transpose(0, 1, 3, 2)
                for dx in range(3):
                    wts[0:64, 3 * c + dx, :] = wc[:, :, 0, dx].T
                    wts[64:128, 3 * c + dx, :] = wc[:, :, 1, dx].T
                wts[0:64, 9 + c, :] = wc[:, :, 2, 0].T
                wts[64:128, 9 + c, :] = wc[:, :, 2, 1].T
                # dy2 dx2 weights in BOTH halves: the K=64 leftover matmuls
                # read it from either row half (paired for concurrency)
                wts[0:64, 12 + c, :] = wc[:, :, 2, 2].T
                wts[64:128, 12 + c, :] = wc[:, :, 2, 2].T
                # biases in (half, chan) layout, separate per scan group
                bg = Bs[tag][ch]
                consts[0:64, c] = bg[0:64]
                consts[64:128, c] = bg[0:64]
                consts[0:64, 3 + c] = bg[64:128]
                consts[64:128, 3 + c] = bg[64:128]
            consts[0:64, 6] = init_a
            consts[64:128, 6] = init_a
            consts[0:64, 7] = init_b
            consts[64:128, 7] = init_b
            cnp = mybir.dt.np(CONV_DT)
            if xpad.dtype != cnp:
                xpad = xpad.astype(cnp)
                wts = wts.astype(cnp)
            in_maps.append(
                {
                    "xp": np.ascontiguousarray(xpad),
                    "wts": wts,
                    "consts": consts,
                }
            )
    return in_maps


def gather_output(core_outs, B, H, W):
    """core_outs: list of 8 arrays (128, H*W) in core order (b-major).

    Device layout: partitions = (half hh in {0,1}, chan c in 0..63);
    free = [band][group g in {0,1}][row r in 0..Rh-1][col]. Global row of
    an element = band*R + hh*Rh + r.
    """
    R, Rh = _R, _R // 2
    nb = H // R
    out = np.empty((B, 64, H, W), np.float32)
    for b in range(B):
        for orient in (0, 1):
            o = core_outs[2 * b + orient].astype(np.float32)
            o = o.reshape(2, 64, nb, 2, Rh, W)  # hh, c, band, g, r, w
            o = o.sum(axis=3)  # sum the two scan groups: hh, c, band, r, w
            o = o.transpose(1, 2, 0, 3, 4).reshape(64, H, W)
            if orient == 0:
                out[b] = o
            else:
                out[b] += o.transpose(0, 2, 1)
    return out


def kernel(**inputs):
    from concourse.bass_utils import run_bass_kernel_spmd

    _ensure_axon_hooks_importable()
    xs = inputs["xs"]
    B, C, H, W = xs.shape
    # the scan-init fixup ops are only needed for nonzero initial states
    # (the problem spec ships all-zero inits)
    need_fixup = any(
        np.any(np.asarray(inputs[k], dtype=np.float32))
        for k in ("h20", "h21", "h30", "h31")
    )
    nc = _get_nc(H, W, with_init_fixup=need_fixup)
    in_maps = make_in_maps(inputs, H, W)
    res = run_bass_kernel_spmd(nc, in_maps, core_ids=list(range(len(in_maps))))
    outs = [res.results[c]["out"] for c in range(len(in_maps))]
    return gather_output(outs, B, H, W)
